# revision 28
# baseline (speedup 1.0000x reference)
"""Trainium2 Bass kernel for nn_MicroStreamBlock (dual-stream block:
quaternion attention branch + Hamilton-mix MLP branch).

Contract: kernel(**inputs) takes the FULL unsharded inputs (as produced by
reference.setup_inputs) and returns the FULL (4, 2048, 2048) float32 output.
Internally the flattened (8192, 2048) token stream is row-sharded across 8
NeuronCores (1024 rows each; a core pair shares one batch).  The per-batch
softmax-over-time partial sums are combined with a tiny pairwise on-device
AllReduce that overlaps with the MLP-branch GEMMs.

Key structure (v3 — dense PE stream, component-major quaternions):
- ALL LayerNorm statistics are computed on the host.  x1 arrives fully
  normalized (bf16, feature-major) for the Hamilton branch; x2 arrives
  fully normalized in fp8 DoubleRow layout for the qkv GEMM.  The
  per-token LN scale of x2 cancels in the quaternion cosine for q/k and
  is exactly right for v; the fp8 weight prescale (64) is undone by
  folding 1/64 into the out-projection weights.
- The q/k/v projection weight columns are permuted on the host so the
  quaternion component index is the OUTER feature axis (c-major).  The
  group-of-4 sums of the quaternion chain then become contiguous
  [128,256] bf16 adds (DVE 2x mode) instead of 1x-mode strided reduces,
  and the v*weight broadcast becomes 4 contiguous slice muls.
- The PE instruction stream is one dense run: qkv (fp8 DR) -> Hamilton
  GEMM1 -> GEMM2 -> attention tail.  Eviction engines: q,k on scalar,
  v on vector; nothing else runs on scalar during the qkv phase, so
  PSUM recycling never stalls the PE (avoids HAM cold-clock
  oscillation).  Activation table loads are batched: Gelu / Ln / Exp /
  Gelu (4 loads total).
- The attention weighted-value tiles are accumulated over token chunks
  on gpsimd; the softmax numerator/denominator column sums are 3 brief
  matmuls, then a pairwise AllReduce overlaps late GEMM1 + GEMM2.
"""

import math
import sys

sys.path.insert(0, "/opt/trn_rl_repo")

import ml_dtypes
import numpy as np

import concourse.bass as bass  # noqa: F401
import concourse.mybir as mybir
import concourse.tile as tile
from concourse import bacc
from concourse.bass_utils import run_bass_kernel_spmd

BF16 = ml_dtypes.bfloat16
F32 = mybir.dt.float32
BF = mybir.dt.bfloat16
AF = mybir.ActivationFunctionType
ALU = mybir.AluOpType
AX = mybir.AxisListType

NCORES = 8
B, T, DIM = 4, 2048, 2048
HALF = DIM // 2          # 1024
HEADS, RANK = 4, 8
NQ = (HALF // HEADS) // 4  # 64
GRP = HEADS * NQ           # 256 quaternion groups per token
ROWS = (B * T) // NCORES   # 1024 rows per core
P = 128
KC = HALF // P             # 8 contraction chunks of 128
TC = ROWS // P             # 8 token chunks of 128
LN_EPS = 1e-5
QEPS = 1e-12               # guard for ln of the quat-norm product
WSCALE = 64.0              # host fp8 weight pre-scale

_CACHE: dict = {}
_LAST_RESULTS = None


def _build_program(with_bias: bool):
    nc = bacc.Bacc("TRN2", target_bir_lowering=False, debug=False,
                   num_devices=NCORES)

    FP8 = mybir.dt.float8e4
    NP = KC // 2  # number of DoubleRow k-pair tiles
    xc = nc.dram_tensor("xc", [ROWS, DIM], F32, kind="ExternalInput").ap()
    xT = nc.dram_tensor("xT", [P, KC, ROWS], FP8, kind="ExternalInput").ap()  # x1n
    # fp8 operands for the qkv GEMM (DoubleRow), one dram tensor per
    # 256-row contraction pair so the first matmuls start after ~1MB
    xdr_d = [nc.dram_tensor(f"xdr{p}", [P, 2, ROWS], FP8,
                            kind="ExternalInput").ap() for p in range(NP)]
    wdr_d = [nc.dram_tensor(f"wdr{p}", [P, 2, 3 * HALF], FP8,
                            kind="ExternalInput").ap() for p in range(NP)]
    f1_d = nc.dram_tensor("f1w", [P, KC, HALF], FP8, kind="ExternalInput").ap()
    f2_d = nc.dram_tensor("f2w", [P, KC, HALF], BF, kind="ExternalInput").ap()
    woT_d = nc.dram_tensor("woT", [P, KC, HALF], BF, kind="ExternalInput").ap()
    b1_d = nc.dram_tensor("b1e", [P, KC], F32, kind="ExternalInput").ap()
    id_d = nc.dram_tensor("ident", [P, P], F32, kind="ExternalInput").ap()
    if with_bias:
        bqkv_d = nc.dram_tensor("bqkve", [1, 3 * HALF], BF, kind="ExternalInput").ap()
        b2_d = nc.dram_tensor("b2e", [1, HALF], BF, kind="ExternalInput").ap()
        bo_d = nc.dram_tensor("boe", [1, HALF], BF, kind="ExternalInput").ap()
    out = nc.dram_tensor("out", [ROWS, DIM], F32, kind="ExternalOutput").ap()

    with tile.TileContext(nc) as tc:
        with tc.tile_pool(name="sb", bufs=1) as sb, \
             tc.tile_pool(name="ps", bufs=1, space="PSUM") as ps, \
             tc.tile_pool(name="dp", bufs=1, space="DRAM") as dp:

            # ---------------- constants ----------------
            ones_bf = sb.tile([P, P], BF, tag="ones_bf")
            nc.vector.memset(ones_bf, 1.0)
            epsq = sb.tile([P, 1], F32, tag="epsq")
            nc.vector.memset(epsq, QEPS)
            warm = sb.tile([P, 1], F32, tag="warm")
            nc.scalar.activation(warm, epsq, AF.Ln)

            # ---------------- loads -------------------
            # critical fp8 qkv operands: xdr pairs on the scalar queue,
            # wdr pairs on the sync queue (two queues pull concurrently)
            xdr = []
            wdr = []
            for p in range(NP):
                tx = sb.tile([P, 2, ROWS], FP8, tag="xdr", bufs=NP,
                             name=f"xdr{p}")
                xdr.append(tx)
                tw = sb.tile([P, 2, 3 * HALF], FP8, tag="wdr", bufs=NP,
                             name=f"wdr{p}")
                wdr.append(tw)
            # alternate queues so each pair lands as early as possible
            nc.scalar.dma_start(out=xdr[0], in_=xdr_d[0])
            nc.sync.dma_start(out=wdr[0], in_=wdr_d[0])
            nc.sync.dma_start(out=xdr[1], in_=xdr_d[1])
            nc.scalar.dma_start(out=wdr[1], in_=wdr_d[1])
            nc.scalar.dma_start(out=xdr[2], in_=xdr_d[2])
            nc.sync.dma_start(out=wdr[2], in_=wdr_d[2])
            nc.sync.dma_start(out=xdr[3], in_=xdr_d[3])
            nc.scalar.dma_start(out=wdr[3], in_=wdr_d[3])
            # bulk weights/x1n: single contiguous DMAs (x1/f1 on scalar
            # behind the xdr pairs, f2/b1 on sync behind the wdr pairs)
            x1big = sb.tile([P, KC, ROWS], FP8, tag="xt", bufs=1, name="x1big")
            nc.scalar.dma_start(out=x1big, in_=xT)
            f1big = sb.tile([P, KC, HALF], FP8, tag="wf", bufs=1, name="f1big")
            nc.sync.dma_start(out=f1big, in_=f1_d)
            f2big = sb.tile([P, KC, HALF], BF, tag="wg", bufs=1, name="f2big")
            nc.sync.dma_start(out=f2big, in_=f2_d)
            f2_t = [f2big[:, k, :] for k in range(KC)]
            b1cols = sb.tile([P, KC], F32, tag="b1cols")
            nc.sync.dma_start(out=b1cols, in_=b1_d)
            ident = sb.tile([P, P], F32, tag="ident")
            nc.sync.dma_start(out=ident, in_=id_d)
            if with_bias:
                bqkvr = sb.tile([1, 3 * HALF], BF, tag="bqkvr")
                nc.sync.dma_start(out=bqkvr, in_=bqkv_d)
                b2r = sb.tile([1, HALF], BF, tag="b2r")
                nc.sync.dma_start(out=b2r, in_=b2_d)
                bor = sb.tile([1, HALF], BF, tag="bor")
                nc.sync.dma_start(out=bor, in_=bo_d)

            # ---------------- stage 1: qkv GEMM (dense fp8-DR stream) ------
            # component-major layout: q = [q0|q1|q2|q3], each [P, GRP]
            vs = [None] * TC
            nrms = [None] * TC
            sqks = [None] * TC
            xn1s = [sb.tile([P, HALF], F32, tag="xn1", bufs=8,
                            name=f"xn1_{tcg}") for tcg in range(TC)]

            for c in range(TC):
                q = sb.tile([P, HALF], BF, tag="qk", bufs=3, name=f"q{c}")
                kk_t = sb.tile([P, HALF], BF, tag="qk", bufs=3, name=f"k{c}")
                v = sb.tile([P, HALF], BF, tag="vv", bufs=8, name=f"v{c}")
                vs[c] = v
                dests = [(q, 0), (q, 512), (kk_t, 0), (kk_t, 512),
                         (v, 0), (v, 512)]
                for jg in range(2):
                    pms = [ps.tile([P, 512], F32, tag="pA", bufs=8,
                                   name=f"pqkv{c}_{jg}_{jj}")
                           for jj in range(3)]
                    for kb in range(NP):
                        for jj in range(3):
                            j = jg * 3 + jj
                            nc.tensor.matmul(
                                pms[jj],
                                lhsT=xdr[kb][:, :, c * P:(c + 1) * P],
                                rhs=wdr[kb][:, :, j * 512:(j + 1) * 512],
                                start=(kb == 0),
                                stop=(kb == NP - 1 and not with_bias),
                                perf_mode=mybir.MatmulPerfMode.DoubleRow)
                    for jj in range(3):
                        j = jg * 3 + jj
                        if with_bias:
                            nc.tensor.matmul(
                                pms[jj],
                                lhsT=ones_bf[0:1, :],
                                rhs=bqkvr[0:1, j * 512:(j + 1) * 512],
                                start=False, stop=True)
                        dt, off = dests[j]
                        # plain copies: LN scale baked into the fp8 operand
                        nc.scalar.copy(dt[:, off:off + 512], pms[jj])

                # quaternion chain (DVE, all contiguous [P,GRP] bf16 ops):
                # sqq = sum_c q_c^2, skk, sqk; nrm = sqq*skk (gpsimd)
                def gsum(a, b, nm, stag="sq2", sbufs=3):
                    prod = sb.tile([P, HALF], BF, tag="pr", bufs=1,
                                   name=f"pr{nm}{c}")
                    nc.vector.tensor_mul(prod, a, b)
                    t0 = sb.tile([P, GRP], BF, tag="tt", bufs=2,
                                 name=f"t0{nm}{c}")
                    nc.vector.tensor_add(t0, prod[:, 0:GRP], prod[:, GRP:2 * GRP])
                    t1 = sb.tile([P, GRP], BF, tag="tt", bufs=2,
                                 name=f"t1{nm}{c}")
                    nc.vector.tensor_add(t1, prod[:, 2 * GRP:3 * GRP],
                                         prod[:, 3 * GRP:4 * GRP])
                    s = sb.tile([P, GRP], BF, tag=stag, bufs=sbufs,
                                name=f"s{nm}{c}")
                    nc.vector.tensor_add(s, t0, t1)
                    return s

                sqq = gsum(q, q, "q")
                skk = gsum(kk_t, kk_t, "k")
                sqk = gsum(q, kk_t, "x", stag="sqk", sbufs=8)
                sqks[c] = sqk
                nrm = sb.tile([P, GRP], BF, tag="nrm", bufs=8, name=f"nrm{c}")
                nc.gpsimd.tensor_mul(nrm, sqq, skk)
                nrms[c] = nrm
                if 1 <= c <= 4:
                    # y1 residual preloads ride the sync queue mid-stream
                    for tcg in (2 * (c - 1), 2 * (c - 1) + 1):
                        nc.sync.dma_start(
                            out=xn1s[tcg],
                            in_=xc[tcg * P:(tcg + 1) * P, 0:HALF])

            # ---------------- stage 2a: GEMM1, quat ACT batches, nd --------
            gts = []
            for jc in range(KC):
                gt = sb.tile([P, ROWS], BF, tag="gt", bufs=8, name=f"gt{jc}")
                gts.append(gt)

            def g1_block(jc):
                for tt in range(2):
                    pm = ps.tile([P, 512], F32, tag="pA", bufs=8,
                                 name=f"pg1_{jc}_{tt}")
                    for kb in range(NP):
                        nc.tensor.matmul(
                            pm,
                            lhsT=f1big[:, 2 * kb:2 * kb + 2,
                                       jc * P:(jc + 1) * P],
                            rhs=x1big[:, 2 * kb:2 * kb + 2,
                                      tt * 512:(tt + 1) * 512],
                            start=(kb == 0), stop=(kb == NP - 1),
                            perf_mode=mybir.MatmulPerfMode.DoubleRow)
                    nc.scalar.activation(gts[jc][:, tt * 512:(tt + 1) * 512],
                                         pm, AF.Gelu, scale=1.0 / WSCALE,
                                         bias=b1cols[:, jc:jc + 1])

            wds = [None] * TC
            wdaccs = [sb.tile([P, HALF + GRP], BF, tag="wdacc", bufs=2,
                              name=f"wdacc{i}") for i in range(2)]

            def quat_tail(cs, acc, eng):
                """batched ACT work (ln set, then exp set) + wd + wdacc"""
                lgs = {}
                for c in cs:
                    lg = sb.tile([P, GRP], F32, tag="lg", bufs=4,
                                 name=f"lg{c}")
                    nc.scalar.activation(lg, nrms[c], AF.Ln, bias=epsq)
                    lgs[c] = lg
                rss = {}
                for c in cs:
                    rs = sb.tile([P, GRP], BF, tag="rs", bufs=3,
                                 name=f"rs{c}")
                    nc.scalar.activation(rs, lgs[c], AF.Exp, scale=-0.5)
                    rss[c] = rs
                coss = {}
                for c in cs:
                    cosn = sb.tile([P, GRP], BF, tag="cos", bufs=3,
                                   name=f"cos{c}")
                    nc.vector.tensor_mul(cosn, sqks[c], rss[c])
                    coss[c] = cosn
                for c in cs:
                    wd = sb.tile([P, HALF + GRP], BF, tag="wd", bufs=2,
                                 name=f"wd{c}")
                    wds[c] = wd
                    nc.scalar.activation(wd[:, HALF:], coss[c], AF.Exp,
                                         scale=1.0 / math.sqrt(NQ))
                    for cc in range(4):
                        nc.vector.tensor_mul(
                            wd[:, cc * GRP:(cc + 1) * GRP],
                            vs[c][:, cc * GRP:(cc + 1) * GRP],
                            wd[:, HALF:])
                    if c == cs[0]:
                        eng.tensor_copy(acc, wd)
                    else:
                        eng.tensor_add(acc, acc, wd)

            # scalar order: LnA ExpA wdExpA | LnB ExpB wdExpB | gelu(0..7)
            # (waves first: scalar is idle at GEMM1 start and the collective
            # input is ready ~14us in, hiding the CC latency under GEMM2)
            quat_tail([0, 1, 2, 3], wdaccs[0], nc.vector)
            g1_block(0)
            quat_tail([4, 5, 6, 7], wdaccs[1], nc.vector)
            g1_block(1)
            g1_block(2)
            g1_block(3)
            # nd column sums (6 brief matmuls) + pairwise AllReduce kickoff
            nd = [ps.tile([1, 512], F32, tag="pA", bufs=8, name="nd0"),
                  ps.tile([1, 512], F32, tag="pA", bufs=8, name="nd1"),
                  ps.tile([1, 256], F32, tag="pA", bufs=8, name="nd2")]
            nd_slices = [(0, 512), (512, 512), (1024, 256)]
            ndrow = sb.tile([1, HALF + GRP], F32, tag="ndrow")
            for s, (lo, n) in enumerate(nd_slices):
                nc.tensor.matmul(nd[s], lhsT=ones_bf[:, 0:1],
                                 rhs=wdaccs[0][:, lo:lo + n],
                                 start=True, stop=False)
                nc.tensor.matmul(nd[s], lhsT=ones_bf[:, 0:1],
                                 rhs=wdaccs[1][:, lo:lo + n],
                                 start=False, stop=True)
                nc.scalar.copy(ndrow[0:1, lo:lo + n], nd[s])
            ndin = dp.tile([1, HALF + GRP], F32, tag="ndin")
            ndout = dp.tile([1, HALF + GRP], F32, tag="ndout")
            nc.scalar.dma_start(out=ndin, in_=ndrow)
            nc.gpsimd.collective_compute(
                "AllReduce", ALU.add,
                replica_groups=[[0, 1], [2, 3], [4, 5], [6, 7]],
                ins=[ndin.opt()], outs=[ndout.opt()])
            ndred = sb.tile([1, HALF + GRP], F32, tag="ndred")
            nc.gpsimd.dma_start(out=ndred, in_=ndout)
            g1_block(4)
            g1_block(5)
            g1_block(6)
            g1_block(7)

            # ---------------- stage 2b: GEMM2 + y2 residual ----------------
            xn2s = {}

            def xn2_load(tcg):
                xn2 = sb.tile([P, HALF], F32, tag="xn", bufs=2,
                              name=f"xn2_{tcg}")
                nc.sync.dma_start(out=xn2,
                                  in_=xc[tcg * P:(tcg + 1) * P, HALF:DIM])
                xn2s[tcg] = xn2

            def g2_block(tcg):
                if tcg + 2 < TC:
                    xn2_load(tcg + 2)
                xn2 = xn2s[tcg]
                yev = sb.tile([P, HALF], F32, tag="wdr", bufs=NP,
                              name=f"yev_{tcg}")
                for jj in range(2):
                    pm = ps.tile([P, 512], F32, tag="pA", bufs=8,
                                 name=f"pg2_{tcg}_{jj}")
                    for k in range(KC):
                        nc.tensor.matmul(
                            pm, lhsT=gts[k][:, tcg * P:(tcg + 1) * P],
                            rhs=f2_t[k][:, jj * 512:(jj + 1) * 512],
                            start=(k == 0), stop=False)
                    if with_bias:
                        nc.tensor.matmul(
                            pm, lhsT=ones_bf[0:1, :],
                            rhs=b2r[0:1, jj * 512:(jj + 1) * 512],
                            start=False, stop=False)
                    # residual folded into the accumulation on the PE
                    nc.tensor.matmul(
                        pm, lhsT=ident,
                        rhs=xn2[:, jj * 512:(jj + 1) * 512],
                        start=False, stop=True)
                    nc.scalar.copy(yev[:, jj * 512:(jj + 1) * 512], pm)
                nc.scalar.dma_start(out=out[tcg * P:(tcg + 1) * P, HALF:DIM],
                                    in_=yev)

            xn2_load(0)
            xn2_load(1)
            for tcg in range(6):
                g2_block(tcg)
            # out-proj weights reuse the f1 memory (free after GEMM1);
            # loaded on the sync queue once the slot-wait is long past
            wobig = sb.tile([P, KC, HALF], BF, tag="wf", bufs=1, name="wobig")
            nc.sync.dma_start(out=wobig, in_=woT_d)
            wo_t = [wobig[:, k, :] for k in range(KC)]
            g2_block(6)

            # ---------------- attention tail: vw, out-proj ------------------
            # placed before the last GEMM2 block: the collective result is
            # normally long since landed; one block of cushion remains.
            rec = sb.tile([1, GRP], F32, tag="rec")
            nc.vector.reciprocal(rec, ndred[0:1, HALF:])
            vw_bf = sb.tile([1, HALF], BF, tag="vwbf")
            nc.vector.tensor_mul(
                vw_bf.rearrange("p (c g) -> p c g", g=GRP),
                ndred[0:1, 0:HALF].rearrange("p (c g) -> p c g", g=GRP),
                rec[0:1, None, :].to_broadcast([1, 4, GRP]))
            vwc = sb.tile([P, KC], BF, tag="vwc")
            for k in range(KC):
                pt = ps.tile([P, 1], F32, tag="pA", bufs=8, name=f"pvw{k}")
                nc.tensor.matmul(pt, lhsT=vw_bf[0:1, k * P:(k + 1) * P],
                                 rhs=ones_bf[0:1, 0:1], start=True, stop=True)
                nc.scalar.copy(vwc[:, k:k + 1], pt)
            orow = sb.tile([1, HALF], BF, tag="orow")
            for h in range(2):
                pm = ps.tile([1, 512], F32, tag="pA", bufs=8, name=f"po{h}")
                for k in range(KC):
                    nc.tensor.matmul(pm, lhsT=vwc[:, k:k + 1],
                                     rhs=wo_t[k][:, h * 512:(h + 1) * 512],
                                     start=(k == 0),
                                     stop=(not with_bias and k == KC - 1))
                if with_bias:
                    nc.tensor.matmul(pm, lhsT=ones_bf[0:1, 0:1],
                                     rhs=bor[0:1, h * 512:(h + 1) * 512],
                                     start=False, stop=True)
                nc.scalar.copy(orow[0:1, h * 512:(h + 1) * 512], pm)
            # broadcast out_row to 128 partitions
            obc = sb.tile([P, HALF], F32, tag="obc", name="obc")
            for h in range(2):
                pb = ps.tile([P, 512], F32, tag="pA", bufs=8, name=f"pbc{h}")
                nc.tensor.matmul(pb, lhsT=ones_bf[0:1, :],
                                 rhs=orow[0:1, h * 512:(h + 1) * 512],
                                 start=True, stop=True)
                nc.scalar.copy(obc[:, h * 512:(h + 1) * 512], pb)

            g2_block(7)

            # y1 = x1 + out (residual rows preloaded during stage 1)
            for tcg in range(TC):
                eng = nc.vector if tcg % 2 == 0 else nc.gpsimd
                eng.tensor_add(xn1s[tcg], xn1s[tcg], obc)
                dma_eng = nc.scalar if tcg % 2 == 0 else nc.sync
                dma_eng.dma_start(out=out[tcg * P:(tcg + 1) * P, 0:HALF],
                                  in_=xn1s[tcg])

    nc.compile()
    return nc


def _get_program(with_bias: bool):
    key = ("nc", with_bias)
    if key not in _CACHE:
        _CACHE[key] = _build_program(with_bias)
    return _CACHE[key]


# permutation that makes the quaternion component index the outer axis:
# perm[c*GRP + g] = g*4 + c
_QPERM = np.arange(HALF).reshape(GRP, 4).T.reshape(-1)


def kernel(**inputs) -> np.ndarray:
    x = np.asarray(inputs["x"], np.float32)
    n1_g = np.asarray(inputs["n1_g"], np.float32)
    n1_b = np.asarray(inputs["n1_b"], np.float32)
    wq = np.asarray(inputs["wq"], np.float32)
    bq = np.asarray(inputs["bq"], np.float32)
    wk = np.asarray(inputs["wk"], np.float32)
    bk = np.asarray(inputs["bk"], np.float32)
    wv = np.asarray(inputs["wv"], np.float32)
    bv = np.asarray(inputs["bv"], np.float32)
    wo = np.asarray(inputs["wo"], np.float32)
    bo = np.asarray(inputs["bo"], np.float32)
    n2_g = np.asarray(inputs["n2_g"], np.float32)
    n2_b = np.asarray(inputs["n2_b"], np.float32)
    f1 = np.asarray(inputs["f1"], np.float32)
    b1 = np.asarray(inputs["b1"], np.float32)
    f2 = np.asarray(inputs["f2"], np.float32)
    b2 = np.asarray(inputs["b2"], np.float32)

    isr = 1.0 / math.sqrt(RANK)
    # fold LN affine: gamma into weight rows, beta into effective bias rows
    F1s = f1.sum(0)
    F2s = f2.sum(0)
    W1 = (n2_g[:, None] * F1s) * isr
    b1e = (n2_b @ F1s) * isr + b1
    # q/k/v projections with output columns permuted to component-major
    Wq = (n1_g[:, None] * wq.T)[:, _QPERM]
    Wk = (n1_g[:, None] * wk.T)[:, _QPERM]
    Wv = (n1_g[:, None] * wv.T)[:, _QPERM]
    Wqkv = np.concatenate([Wq, Wk, Wv], axis=1)
    bqkve = np.concatenate([(n1_b @ wq.T + bq)[_QPERM],
                            (n1_b @ wk.T + bk)[_QPERM],
                            (n1_b @ wv.T + bv)[_QPERM]])

    with_bias = bool(np.any(bqkve) or np.any(b2) or np.any(bo))

    FP8 = np.dtype(mybir.dt.np(mybir.dt.float8e4))

    def chunked(a):
        # [HALF, X] -> [P, KC, X] with row = kc*128 + pi
        return np.ascontiguousarray(
            a.reshape(KC, P, a.shape[1]).transpose(1, 0, 2))

    f1_q = chunked((W1 * WSCALE).astype(BF16).astype(np.float32)
                   .astype(np.dtype(mybir.dt.np(mybir.dt.float8e4))))
    f2_bf = chunked((F2s * isr).astype(BF16))
    # wo rows permuted to match the component-major vw; fold in the fp8
    # weight prescale undo (1/WSCALE)
    woT_bf = chunked((wo.T[_QPERM] * (1.0 / WSCALE)).astype(BF16))
    # qkv weights: scale by WSCALE for fp8 resolution, interleave d=po*128+pi
    wdr = np.ascontiguousarray(
        (Wqkv * WSCALE).reshape(KC, P, 3 * HALF).transpose(1, 0, 2)).astype(FP8)

    xf = np.ascontiguousarray(x.reshape(B * T, DIM))
    shared = {
        "f1w": f1_q,
        "f2w": f2_bf,
        "woT": woT_bf,
        "b1e": np.ascontiguousarray(
            b1e.reshape(KC, P).T, dtype=np.float32),
        "ident": np.eye(P, dtype=np.float32),
    }
    for p in range(KC // 2):
        shared[f"wdr{p}"] = np.ascontiguousarray(wdr[:, 2 * p:2 * p + 2, :])
    if with_bias:
        shared["bqkve"] = np.ascontiguousarray(
            WSCALE * bqkve.reshape(1, -1)).astype(BF16)
        shared["b2e"] = np.ascontiguousarray(b2.reshape(1, -1)).astype(BF16)
        shared["boe"] = np.ascontiguousarray(bo.reshape(1, -1)).astype(BF16)
    in_maps = []
    for i in range(NCORES):
        rows = xf[i * ROWS:(i + 1) * ROWS]
        m = dict(shared)
        m["xc"] = rows
        # branch-a LN fully on host: x1n^T in bf16 (affine folded into f1/b1)
        x1 = rows[:, :HALF].T.astype(np.float32)          # [d, tok]
        mu1 = x1.mean(0, keepdims=True)
        x1c = x1 - mu1
        istd1 = 1.0 / np.sqrt((x1c * x1c).mean(0, keepdims=True) + LN_EPS)
        m["xT"] = np.ascontiguousarray(
            (x1c * istd1).astype(np.dtype(mybir.dt.np(mybir.dt.float8e4)))
            .reshape(KC, P, ROWS).transpose(1, 0, 2))
        # branch-b LN fully on host inside the fp8 qkv operand: the
        # per-token scale cancels for q/k and is exactly right for v
        x2 = rows[:, HALF:].T.astype(np.float32)          # [d, tok]
        mu2 = x2.mean(0, keepdims=True)
        x2c = x2 - mu2
        istd2 = 1.0 / np.sqrt((x2c * x2c).mean(0, keepdims=True) + LN_EPS)
        xq = (x2c * istd2).astype(FP8).reshape(KC, P, ROWS)
        for p in range(KC // 2):
            m[f"xdr{p}"] = np.ascontiguousarray(
                xq[2 * p:2 * p + 2].transpose(1, 0, 2))
        in_maps.append(m)

    nc = _get_program(with_bias)
    res = run_bass_kernel_spmd(nc, in_maps, core_ids=list(range(NCORES)))
    global _LAST_RESULTS
    _LAST_RESULTS = res
    y = np.concatenate([res.results[i]["out"] for i in range(NCORES)], axis=0)
    return np.ascontiguousarray(y.reshape(B, T, DIM))
